# revision 61
# baseline (speedup 1.0000x reference)
"""Trainium2 Bass kernel for nn_AttentionNet (Bahdanau-style attention pooling).

Computation (reference):
    attn1 = enc_out @ W_enc + b_enc              # [B, L, A]
    attn2 = dec_hidden @ W_dec + b_dec           # [B, A]
    attn  = relu(attn1 + attn2[:, None]) @ W_full + b_full   # [B, L]
    alpha = softmax(attn, axis=1)                # [B, L]
    awe   = einsum("ble,bl->be", enc_out, alpha) # [B, E]
    returns (awe, alpha)

Sharding: data-parallel over batch B across 8 NeuronCores (8 batches/core).
Weights replicated. b_full is dropped (softmax shift-invariant).

Per-core dataflow ([a, l] orientation so bias+relu fuse into one ScalarE
activation with per-partition bias):
  - enc shard is DMA'd once into SBUF as float32r (SWDGE cast rounds),
    resident for both passes.
  - PE transposes 128x128 blocks of enc -> encT (e on partitions).
  - attn1^T: 4x4 chunked fp32r matmuls, stationary = W_enc chunk.
  - bias+relu fused on ScalarE: relu(Z + (b_enc + b_dec + attn2[b])[a]).
  - scores: fp32r matmul with W_full chunk stationary, accumulated over a.
  - per-batch softmax (no max-subtraction; scores are O(1)): exp fused
    into the scores PSUM->SBUF copy; [1, L] row spread into [128, L/128]
    columns with K=1 matmuls against [[1]]; free-dim reduce + ones-matmul
    partition reduce for the denominator.
  - pass 2 (interleaved per batch): awe = sum_l alpha[l] * enc[l, :] as
    fp32r matmuls with the alpha column stationary and the resident
    natural-layout enc tiles moving.
"""

import numpy as np

import concourse.bacc as bacc
import concourse.mybir as mybir
import concourse.tile as tile
from concourse import bass_utils

F32 = mybir.dt.float32
F32R = mybir.dt.float32r
AX = mybir.AxisListType
OP = mybir.AluOpType
ACT = mybir.ActivationFunctionType

B, L, E, A, D = 64, 1024, 512, 512, 512
NCORES = 8
BSH = B // NCORES          # 8 batches per core
ROWS = BSH * L             # 8192 rows per core
NT = ROWS // 128           # 64 l-tiles of 128 rows
NB = ROWS // 512           # 16 l-blocks of 512 rows (4 tiles each)
TPB = L // 128             # 8 l-tiles per batch


def _build():
    nc = bacc.Bacc("TRN2", target_bir_lowering=False, debug=False)

    enc = nc.dram_tensor("enc", [BSH, L, E], F32, kind="ExternalInput")
    dec = nc.dram_tensor("dec", [BSH, D], F32, kind="ExternalInput")
    w_enc = nc.dram_tensor("w_enc", [E, A], F32, kind="ExternalInput")
    b_enc = nc.dram_tensor("b_enc", [A], F32, kind="ExternalInput")
    w_dec = nc.dram_tensor("w_dec", [D, A], F32, kind="ExternalInput")
    b_dec = nc.dram_tensor("b_dec", [A], F32, kind="ExternalInput")
    w_full = nc.dram_tensor("w_full", [A], F32, kind="ExternalInput")
    ident_in = nc.dram_tensor("ident_in", [128, 128], F32, kind="ExternalInput")

    awe_out = nc.dram_tensor("awe", [BSH, E], F32, kind="ExternalOutput")
    alpha_out = nc.dram_tensor("alpha_raw", [128, NT], F32, kind="ExternalOutput")

    with tile.TileContext(nc) as tc:
        with (
            tc.tile_pool(name="const", bufs=1) as cpool,
            tc.tile_pool(name="enc", bufs=1) as encpool,
            tc.tile_pool(name="work", bufs=2) as wpool,
            tc.tile_pool(name="zrelu", bufs=6) as zpool,
            tc.tile_pool(name="small", bufs=2) as spool,
            tc.tile_pool(name="et_ps", bufs=3, space="PSUM") as etps,
            tc.tile_pool(name="z_ps", bufs=2, space="PSUM") as zps,
            tc.tile_pool(name="sc_ps", bufs=1, space="PSUM") as scps,
            tc.tile_pool(name="misc_ps", bufs=2, space="PSUM") as mps,
        ):
            # ---------------- constants + enc load ----------------
            w_enc_r = cpool.tile([128, 4 * A], F32R, tag="w_enc_r")
            w_dec_sb = cpool.tile([128, 4 * A], F32, tag="w_dec")
            w_full_r = cpool.tile([128, 4], F32R, tag="w_full_r")
            ident = cpool.tile([128, 128], F32, tag="ident")
            ident_r = cpool.tile([128, 128], F32R, tag="ident_r")
            ones_col = cpool.tile([128, 1], F32, tag="ones_col")
            ones_row = cpool.tile([1, 128], F32, tag="ones_row")
            benc_P = cpool.tile([128, 4], F32, tag="benc_P")
            bdec_P = cpool.tile([128, 4], F32, tag="bdec_P")
            bsum_P = cpool.tile([128, 4], F32, tag="bsum_P")
            dec_sb = cpool.tile([BSH, D], F32, tag="dec")

            # identity comes in as a host-provided input on the (otherwise
            # idle) HWDGE queue so the gpsimd queue stays clear for enc and
            # the scheduler can't push it behind the bulk loads
            nc.sync.dma_start(out=ident[:, :], in_=ident_in.ap())

            enc_view = enc.ap().flatten_outer_dims().rearrange(
                "(t p) e -> p t e", p=128
            )  # [128, 64, 512]
            CH = 2  # l-tiles per DMA chunk (0.5 MiB) — finer-grained deps
            NCH = NT // CH  # 32 chunks
            enc_chunks = []
            for k in range(NCH):
                enc_chunks.append(encpool.tile([128, CH * E], F32R, tag=f"enc{k}", name=f"enc_sb{k}"))

            def enc_tile_ap(t):
                return enc_chunks[t // CH][:, (t % CH) * E : (t % CH + 1) * E]

            def load_chunk(k):
                nc.gpsimd.dma_start(  # SWDGE: casts fp32 -> fp32r (rounds)
                    out=enc_chunks[k][:, :].rearrange("p (t e) -> p t e", e=E),
                    in_=enc_view[:, k * CH : (k + 1) * CH, :],
                )

            load_chunk(0)
            load_chunk(1)
            for ec in range(4):  # cast fp32 -> fp32r, chunked so ec=0 lands first
                nc.gpsimd.dma_start(
                    out=w_enc_r[:, ec * A : (ec + 1) * A],
                    in_=w_enc.ap()[ec * 128 : (ec + 1) * 128, :],
                )
            load_chunk(2)
            load_chunk(3)
            nc.gpsimd.dma_start(  # first scores matmul only needs this at ~8us
                out=w_full_r[:, :], in_=w_full.ap().rearrange("(c p) -> p c", p=128)
            )
            for k in range(4, NCH):
                load_chunk(k)

            # SP (HWDGE) queue: dec first (gates the attn2 chain), w_dec in
            # per-chunk DMAs so each attn2 matmul starts as its chunk lands,
            # bias vectors in fast [128, 4] column layout.
            nc.sync.dma_start(out=dec_sb[:, :], in_=dec.ap())
            for dc in range(4):
                nc.sync.dma_start(
                    out=w_dec_sb[:, dc * A : (dc + 1) * A],
                    in_=w_dec.ap()[dc * 128 : (dc + 1) * 128, :],
                )
            nc.sync.dma_start(
                out=benc_P[:, :], in_=b_enc.ap().rearrange("(c p) -> p c", p=128)
            )
            nc.sync.dma_start(
                out=bdec_P[:, :], in_=b_dec.ap().rearrange("(c p) -> p c", p=128)
            )
            nc.vector.memset(ones_col[:, :], 1.0)
            nc.vector.memset(ones_row[:, :], 1.0)
            nc.vector.tensor_copy(ident_r[:, :], ident[:, :])
            nc.vector.tensor_add(bsum_P[:, :], benc_P[:, :], bdec_P[:, :])
            # fp32r copies for the attn2 matmuls (DVE is idle this early)
            dec_r = cpool.tile([BSH, D], F32R, tag="dec_r")
            wdec_r = cpool.tile([128, 4 * A], F32R, tag="wdec_r")
            nc.vector.tensor_copy(dec_r[:, :], dec_sb[:, :])
            for dc in range(4):
                nc.vector.tensor_copy(
                    wdec_r[:, dc * A : (dc + 1) * A],
                    w_dec_sb[:, dc * A : (dc + 1) * A],
                )

            # ---------------- attn2 + bias ----------------
            # decT: [d, b] layout via PE transposes of dec_r chunks (fp32r)
            decT_sb = cpool.tile([128, 4 * BSH], F32R, tag="decT")
            for dc in range(4):
                tp = mps.tile([128, BSH], F32R, tag="misc", name=f"dtp{dc}")
                nc.tensor.transpose(
                    tp[:, :], dec_r[:, dc * 128 : (dc + 1) * 128],
                    ident_r[0:BSH, 0:BSH],
                )
                nc.vector.tensor_copy(decT_sb[:, dc * BSH : (dc + 1) * BSH], tp[:, :])
            # attn2[b, a] (biases folded in later, during the transposes);
            # kept in fp32r so its transposes can reuse ident_r
            attn2_sb = cpool.tile([BSH, A], F32R, tag="attn2")
            a2ps = mps.tile([BSH, A], F32, tag="misc")
            for dc in range(4):
                nc.tensor.matmul(
                    a2ps[:, :],
                    decT_sb[:, dc * BSH : (dc + 1) * BSH],
                    wdec_r[:, dc * A : (dc + 1) * A],
                    start=(dc == 0), stop=(dc == 3),
                )
            nc.vector.tensor_copy(attn2_sb[:, :], a2ps[:, :])
            # bias_sb[p, ac*8 + b] = attn2_sb[b, ac*128+p] + b_enc[.] + b_dec[.]
            bias_sb = cpool.tile([128, 4 * BSH], F32, tag="bias")
            for ac in range(4):
                tp = mps.tile([128, BSH], F32R, tag="misc", name=f"btp{ac}")
                nc.tensor.transpose(
                    tp[:, :], attn2_sb[:, ac * 128 : (ac + 1) * 128],
                    ident_r[0:BSH, 0:BSH],
                )
                nc.vector.tensor_scalar_add(
                    bias_sb[:, ac * BSH : (ac + 1) * BSH], tp[:, :],
                    bsum_P[:, ac : ac + 1],
                )

            # ------- pass 1 + per-batch softmax + interleaved pass 2 -------
            # pass 2 for batch b is issued after batch b+1's softmax so its
            # matmuls fill the PE idle during the serial softmax chain
            alpha_r = cpool.tile([128, NT], F32R, tag="alpha")

            def do_pass2(b):
                ap_ps = mps.tile([1, E], F32, tag="misc", name=f"awe_ps{b}")
                for c in range(TPB):
                    t = b * TPB + c
                    nc.tensor.matmul(
                        ap_ps[:, :],
                        alpha_r[:, t : t + 1],
                        enc_tile_ap(t),
                        start=(c == 0), stop=(c == TPB - 1),
                    )
                aweb = spool.tile([1, E], F32, tag="aweb", name=f"aweb{b}")
                nc.scalar.copy(aweb[:, :], ap_ps[:, :])
                nc.sync.dma_start(out=awe_out.ap()[b : b + 1, :], in_=aweb[:, :])

            for b in range(BSH):
                sbb = spool.tile([1, 2 * 512], F32, tag="scb")
                expP = mps.tile([128, TPB], F32, tag="misc")
                for half in range(2):
                    lb = 2 * b + half
                    encT = wpool.tile([128, 4 * 512], F32R, tag="encT")
                    for ec in range(4):
                        etp = etps.tile([128, 512], F32R, tag="et")
                        for lt in range(4):
                            t = 4 * lb + lt
                            src = enc_tile_ap(t)
                            nc.tensor.transpose(
                                etp[:, lt * 128 : (lt + 1) * 128],
                                src[:, ec * 128 : ec * 128 + 128],
                                ident_r[:, :],
                            )
                        if ec % 2 == 0:
                            nc.vector.tensor_copy(
                                encT[:, ec * 512 : (ec + 1) * 512], etp[:, :]
                            )
                        else:
                            nc.scalar.copy(
                                encT[:, ec * 512 : (ec + 1) * 512], etp[:, :]
                            )
                    zr = []
                    for ac in range(4):
                        zp = zps.tile([128, 512], F32, tag="z")
                        for ec in range(4):
                            nc.tensor.matmul(
                                zp[:, :],
                                w_enc_r[:, ec * A + ac * 128 : ec * A + ac * 128 + 128],
                                encT[:, ec * 512 : (ec + 1) * 512],
                                start=(ec == 0), stop=(ec == 3),
                            )
                        z = zpool.tile([128, 512], F32R, tag="zr")
                        nc.scalar.activation(
                            z[:, :], zp[:, :], ACT.Relu,
                            bias=bias_sb[:, ac * BSH + b : ac * BSH + b + 1],
                        )
                        zr.append(z)
                    scp = scps.tile([1, 512], F32, tag="sc")
                    for ac in range(4):
                        nc.tensor.matmul(
                            scp[:, :], w_full_r[:, ac : ac + 1], zr[ac][:, :],
                            start=(ac == 0), stop=(ac == 3),
                        )
                    # copy scores PSUM->SBUF with exp fused (softmax without
                    # max-subtraction: scores are O(1) here)
                    nc.scalar.activation(
                        sbb[0:1, half * 512 : (half + 1) * 512], scp[:, :], ACT.Exp
                    )
                    # spread this half's exp-scores into columns [128, 4] via
                    # K=1 matmuls against [[1.0]] (out[m,0] = in[0,m])
                    for j in range(4):
                        c8 = half * 4 + j
                        nc.tensor.matmul(
                            expP[:, c8 : c8 + 1],
                            sbb[0:1, c8 * 128 : (c8 + 1) * 128],
                            ones_row[0:1, 0:1],
                            start=True, stop=True,
                        )
                if b > 0:
                    # previous batch's weighted sum: program-ordered before
                    # this batch's softmax chain so its matmuls (and PSUM
                    # slot) fill the PE idle during the serial chain
                    do_pass2(b - 1)
                partb = spool.tile([128, 1], F32, tag="partb")
                nc.vector.tensor_reduce(partb[:, :], expP[:, :], axis=AX.X, op=OP.add)
                d_ps = mps.tile([1, 1], F32, tag="misc")
                nc.tensor.matmul(
                    d_ps[:, :], ones_col[:, :], partb[:, :], start=True, stop=True
                )
                rd = spool.tile([1, 1], F32, tag="rd")
                nc.vector.reciprocal(rd[:, :], d_ps[:, :])
                r_ps = mps.tile([128, 1], F32, tag="misc")
                nc.tensor.matmul(
                    r_ps[:, :], ones_row[:, :], rd[:, :], start=True, stop=True
                )
                rrepb = spool.tile([128, 1], F32, tag="rrepb")
                nc.vector.tensor_copy(rrepb[:, :], r_ps[:, :])
                nc.vector.tensor_scalar_mul(
                    alpha_r[:, b * TPB : (b + 1) * TPB], expP[:, :], rrepb[:, :]
                )
            do_pass2(BSH - 1)

            nc.sync.dma_start(out=alpha_out.ap(), in_=alpha_r[:, :].bitcast(F32))

    nc.compile()
    return nc


_NC = None


def _get_nc():
    global _NC
    if _NC is None:
        _NC = _build()
    return _NC


def kernel(enc_out, dec_hidden, W_enc, b_enc, W_dec, b_dec, W_full, b_full=None):
    enc_out = np.ascontiguousarray(enc_out, dtype=np.float32)
    dec_hidden = np.ascontiguousarray(dec_hidden, dtype=np.float32)
    in_maps = []
    for c in range(NCORES):
        in_maps.append({
            "enc": enc_out[c * BSH : (c + 1) * BSH],
            "dec": dec_hidden[c * BSH : (c + 1) * BSH],
            "w_enc": np.ascontiguousarray(W_enc, np.float32),
            "b_enc": np.ascontiguousarray(b_enc, np.float32),
            "w_dec": np.ascontiguousarray(W_dec, np.float32),
            "b_dec": np.ascontiguousarray(b_dec, np.float32),
            "w_full": np.ascontiguousarray(W_full, np.float32),
            "ident_in": np.eye(128, dtype=np.float32),
        })
    res = bass_utils.run_bass_kernel_spmd(_get_nc(), in_maps, core_ids=list(range(NCORES)))
    awe = np.concatenate([res.results[c]["awe"] for c in range(NCORES)], axis=0)
    alpha = np.concatenate(
        [
            res.results[c]["alpha_raw"]
            .reshape(128, BSH, TPB)
            .transpose(1, 2, 0)
            .reshape(BSH, L)
            for c in range(NCORES)
        ],
        axis=0,
    )
    return awe, alpha


# revision 67
# speedup vs baseline: 1.0001x; 1.0001x over previous
"""Trainium2 Bass kernel for nn_AttentionNet (Bahdanau-style attention pooling).

Computation (reference):
    attn1 = enc_out @ W_enc + b_enc              # [B, L, A]
    attn2 = dec_hidden @ W_dec + b_dec           # [B, A]
    attn  = relu(attn1 + attn2[:, None]) @ W_full + b_full   # [B, L]
    alpha = softmax(attn, axis=1)                # [B, L]
    awe   = einsum("ble,bl->be", enc_out, alpha) # [B, E]
    returns (awe, alpha)

Sharding: data-parallel over batch B across 8 NeuronCores (8 batches/core).
Weights replicated. b_full is dropped (softmax shift-invariant).

Per-core dataflow ([a, l] orientation so bias+relu fuse into one ScalarE
activation with per-partition bias):
  - enc shard is DMA'd once into SBUF as float32r (SWDGE cast rounds),
    resident for both passes.
  - PE transposes 128x128 blocks of enc -> encT (e on partitions).
  - attn1^T: 4x4 chunked fp32r matmuls, stationary = W_enc chunk.
  - bias+relu fused on ScalarE: relu(Z + (b_enc + b_dec + attn2[b])[a]).
  - scores: fp32r matmul with W_full chunk stationary, accumulated over a.
  - per-batch softmax (no max-subtraction; scores are O(1)): exp fused
    into the scores PSUM->SBUF copy; [1, L] row spread into [128, L/128]
    columns with K=1 matmuls against [[1]]; free-dim reduce + ones-matmul
    partition reduce for the denominator.
  - pass 2 (interleaved per batch): awe = sum_l alpha[l] * enc[l, :] as
    fp32r matmuls with the alpha column stationary and the resident
    natural-layout enc tiles moving.
"""

import numpy as np

import concourse.bacc as bacc
import concourse.mybir as mybir
import concourse.tile as tile
from concourse import bass_utils

F32 = mybir.dt.float32
F32R = mybir.dt.float32r
AX = mybir.AxisListType
OP = mybir.AluOpType
ACT = mybir.ActivationFunctionType

B, L, E, A, D = 64, 1024, 512, 512, 512
NCORES = 8
BSH = B // NCORES          # 8 batches per core
ROWS = BSH * L             # 8192 rows per core
NT = ROWS // 128           # 64 l-tiles of 128 rows
NB = ROWS // 512           # 16 l-blocks of 512 rows (4 tiles each)
TPB = L // 128             # 8 l-tiles per batch


def _build():
    nc = bacc.Bacc("TRN2", target_bir_lowering=False, debug=False)

    enc = nc.dram_tensor("enc", [BSH, L, E], F32, kind="ExternalInput")
    dec = nc.dram_tensor("dec", [BSH, D], F32, kind="ExternalInput")
    w_enc = nc.dram_tensor("w_enc", [E, A], F32, kind="ExternalInput")
    b_enc = nc.dram_tensor("b_enc", [A], F32, kind="ExternalInput")
    w_dec = nc.dram_tensor("w_dec", [D, A], F32, kind="ExternalInput")
    b_dec = nc.dram_tensor("b_dec", [A], F32, kind="ExternalInput")
    w_full = nc.dram_tensor("w_full", [A], F32, kind="ExternalInput")
    ident_in = nc.dram_tensor("ident_in", [128, 128], mybir.dt.bfloat16, kind="ExternalInput")

    awe_out = nc.dram_tensor("awe", [BSH, E], F32, kind="ExternalOutput")
    alpha_out = nc.dram_tensor("alpha_raw", [128, NT], F32, kind="ExternalOutput")

    with tile.TileContext(nc) as tc:
        with (
            tc.tile_pool(name="const", bufs=1) as cpool,
            tc.tile_pool(name="enc", bufs=1) as encpool,
            tc.tile_pool(name="work", bufs=2) as wpool,
            tc.tile_pool(name="zrelu", bufs=6) as zpool,
            tc.tile_pool(name="small", bufs=2) as spool,
            tc.tile_pool(name="et_ps", bufs=3, space="PSUM") as etps,
            tc.tile_pool(name="z_ps", bufs=2, space="PSUM") as zps,
            tc.tile_pool(name="sc_ps", bufs=1, space="PSUM") as scps,
            tc.tile_pool(name="misc_ps", bufs=2, space="PSUM") as mps,
        ):
            # ---------------- constants + enc load ----------------
            w_enc_r = cpool.tile([128, 4 * A], F32R, tag="w_enc_r")
            w_dec_sb = cpool.tile([128, 4 * A], F32, tag="w_dec")
            w_full_r = cpool.tile([128, 4], F32R, tag="w_full_r")
            ident_bf = cpool.tile([128, 128], mybir.dt.bfloat16, tag="ident_bf")
            ident_r = cpool.tile([128, 128], F32R, tag="ident_r")
            ones_col = cpool.tile([128, 1], F32, tag="ones_col")
            ones_row = cpool.tile([1, 128], F32, tag="ones_row")
            benc_P = cpool.tile([128, 4], F32, tag="benc_P")
            bdec_P = cpool.tile([128, 4], F32, tag="bdec_P")
            bsum_P = cpool.tile([128, 4], F32, tag="bsum_P")
            dec_sb = cpool.tile([BSH, D], F32, tag="dec")


            enc_view = enc.ap().flatten_outer_dims().rearrange(
                "(t p) e -> p t e", p=128
            )  # [128, 64, 512]
            CH = 2  # l-tiles per DMA chunk (0.5 MiB) — finer-grained deps
            NCH = NT // CH  # 32 chunks
            enc_chunks = []
            for k in range(NCH):
                enc_chunks.append(encpool.tile([128, CH * E], F32R, tag=f"enc{k}", name=f"enc_sb{k}"))

            def enc_tile_ap(t):
                return enc_chunks[t // CH][:, (t % CH) * E : (t % CH + 1) * E]

            def load_chunk(k):
                nc.gpsimd.dma_start(  # SWDGE: casts fp32 -> fp32r (rounds)
                    out=enc_chunks[k][:, :].rearrange("p (t e) -> p t e", e=E),
                    in_=enc_view[:, k * CH : (k + 1) * CH, :],
                )

            # identity (bf16, 32KB) first on the fast-semaphore SWDGE
            # queue: its sem fires ~0.2us after transfer vs ~1.7us HWDGE
            # receipt, unblocking the first PE transposes much earlier
            nc.gpsimd.dma_start(out=ident_bf[:, :], in_=ident_in.ap())
            load_chunk(0)
            load_chunk(1)
            for ec in range(4):  # cast fp32 -> fp32r, chunked so ec=0 lands first
                nc.gpsimd.dma_start(
                    out=w_enc_r[:, ec * A : (ec + 1) * A],
                    in_=w_enc.ap()[ec * 128 : (ec + 1) * 128, :],
                )
            load_chunk(2)
            load_chunk(3)
            nc.gpsimd.dma_start(  # first scores matmul only needs this at ~8us
                out=w_full_r[:, :], in_=w_full.ap().rearrange("(c p) -> p c", p=128)
            )
            for k in range(4, NCH):
                load_chunk(k)

            # SP (HWDGE) queue: dec first (gates the attn2 chain), w_dec in
            # per-chunk DMAs so each attn2 matmul starts as its chunk lands,
            # bias vectors in fast [128, 4] column layout.
            nc.sync.dma_start(out=dec_sb[:, :], in_=dec.ap())
            for dc in range(4):
                nc.sync.dma_start(
                    out=w_dec_sb[:, dc * A : (dc + 1) * A],
                    in_=w_dec.ap()[dc * 128 : (dc + 1) * 128, :],
                )
            nc.sync.dma_start(
                out=benc_P[:, :], in_=b_enc.ap().rearrange("(c p) -> p c", p=128)
            )
            nc.sync.dma_start(
                out=bdec_P[:, :], in_=b_dec.ap().rearrange("(c p) -> p c", p=128)
            )
            nc.vector.memset(ones_col[:, :], 1.0)
            nc.vector.memset(ones_row[:, :], 1.0)
            nc.vector.tensor_copy(ident_r[:, :], ident_bf[:, :])
            nc.vector.tensor_add(bsum_P[:, :], benc_P[:, :], bdec_P[:, :])
            # fp32r copies for the attn2 matmuls (DVE is idle this early)
            dec_r = cpool.tile([BSH, D], F32R, tag="dec_r")
            wdec_r = cpool.tile([128, 4 * A], F32R, tag="wdec_r")
            nc.vector.tensor_copy(dec_r[:, :], dec_sb[:, :])
            for dc in range(4):
                nc.vector.tensor_copy(
                    wdec_r[:, dc * A : (dc + 1) * A],
                    w_dec_sb[:, dc * A : (dc + 1) * A],
                )

            # ---------------- attn2 + bias ----------------
            # decT: [d, b] layout via PE transposes of dec_r chunks (fp32r)
            decT_sb = cpool.tile([128, 4 * BSH], F32R, tag="decT")
            for dc in range(4):
                tp = mps.tile([128, BSH], F32R, tag="misc", name=f"dtp{dc}")
                nc.tensor.transpose(
                    tp[:, :], dec_r[:, dc * 128 : (dc + 1) * 128],
                    ident_r[0:BSH, 0:BSH],
                )
                nc.vector.tensor_copy(decT_sb[:, dc * BSH : (dc + 1) * BSH], tp[:, :])
            # attn2[b, a] (biases folded in later, during the transposes);
            # kept in fp32r so its transposes can reuse ident_r
            attn2_sb = cpool.tile([BSH, A], F32R, tag="attn2")
            a2ps = mps.tile([BSH, A], F32, tag="misc")
            for dc in range(4):
                nc.tensor.matmul(
                    a2ps[:, :],
                    decT_sb[:, dc * BSH : (dc + 1) * BSH],
                    wdec_r[:, dc * A : (dc + 1) * A],
                    start=(dc == 0), stop=(dc == 3),
                )
            nc.vector.tensor_copy(attn2_sb[:, :], a2ps[:, :])
            # bias_sb[p, ac*8 + b] = attn2_sb[b, ac*128+p] + b_enc[.] + b_dec[.]
            bias_sb = cpool.tile([128, 4 * BSH], F32, tag="bias")
            for ac in range(4):
                tp = mps.tile([128, BSH], F32R, tag="misc", name=f"btp{ac}")
                nc.tensor.transpose(
                    tp[:, :], attn2_sb[:, ac * 128 : (ac + 1) * 128],
                    ident_r[0:BSH, 0:BSH],
                )
                nc.vector.tensor_scalar_add(
                    bias_sb[:, ac * BSH : (ac + 1) * BSH], tp[:, :],
                    bsum_P[:, ac : ac + 1],
                )

            # ------- pass 1 + per-batch softmax + interleaved pass 2 -------
            # pass 2 for batch b is issued after batch b+1's softmax so its
            # matmuls fill the PE idle during the serial softmax chain
            alpha_r = cpool.tile([128, NT], F32R, tag="alpha")

            def do_pass2(b):
                ap_ps = mps.tile([1, E], F32, tag="misc", name=f"awe_ps{b}")
                for c in range(TPB):
                    t = b * TPB + c
                    nc.tensor.matmul(
                        ap_ps[:, :],
                        alpha_r[:, t : t + 1],
                        enc_tile_ap(t),
                        start=(c == 0), stop=(c == TPB - 1),
                    )
                aweb = spool.tile([1, E], F32, tag="aweb", name=f"aweb{b}")
                nc.scalar.copy(aweb[:, :], ap_ps[:, :])
                nc.sync.dma_start(out=awe_out.ap()[b : b + 1, :], in_=aweb[:, :])

            for b in range(BSH):
                sbb = spool.tile([1, 2 * 512], F32, tag="scb")
                expP = mps.tile([128, TPB], F32, tag="misc")
                for half in range(2):
                    lb = 2 * b + half
                    encT = wpool.tile([128, 4 * 512], F32R, tag="encT")
                    for ec in range(4):
                        etp = etps.tile([128, 512], F32R, tag="et")
                        for lt in range(4):
                            t = 4 * lb + lt
                            src = enc_tile_ap(t)
                            nc.tensor.transpose(
                                etp[:, lt * 128 : (lt + 1) * 128],
                                src[:, ec * 128 : ec * 128 + 128],
                                ident_r[:, :],
                            )
                        if ec % 2 == 0:
                            nc.vector.tensor_copy(
                                encT[:, ec * 512 : (ec + 1) * 512], etp[:, :]
                            )
                        else:
                            nc.scalar.copy(
                                encT[:, ec * 512 : (ec + 1) * 512], etp[:, :]
                            )
                    zr = []
                    for ac in range(4):
                        zp = zps.tile([128, 512], F32, tag="z")
                        for ec in range(4):
                            nc.tensor.matmul(
                                zp[:, :],
                                w_enc_r[:, ec * A + ac * 128 : ec * A + ac * 128 + 128],
                                encT[:, ec * 512 : (ec + 1) * 512],
                                start=(ec == 0), stop=(ec == 3),
                            )
                        z = zpool.tile([128, 512], F32R, tag="zr")
                        nc.scalar.activation(
                            z[:, :], zp[:, :], ACT.Relu,
                            bias=bias_sb[:, ac * BSH + b : ac * BSH + b + 1],
                        )
                        zr.append(z)
                    scp = scps.tile([1, 512], F32, tag="sc")
                    for ac in range(4):
                        nc.tensor.matmul(
                            scp[:, :], w_full_r[:, ac : ac + 1], zr[ac][:, :],
                            start=(ac == 0), stop=(ac == 3),
                        )
                    # copy scores PSUM->SBUF with exp fused (softmax without
                    # max-subtraction: scores are O(1) here)
                    nc.scalar.activation(
                        sbb[0:1, half * 512 : (half + 1) * 512], scp[:, :], ACT.Exp
                    )
                    # spread this half's exp-scores into columns [128, 4] via
                    # K=1 matmuls against [[1.0]] (out[m,0] = in[0,m])
                    for j in range(4):
                        c8 = half * 4 + j
                        nc.tensor.matmul(
                            expP[:, c8 : c8 + 1],
                            sbb[0:1, c8 * 128 : (c8 + 1) * 128],
                            ones_row[0:1, 0:1],
                            start=True, stop=True,
                        )
                if b > 0:
                    # previous batch's weighted sum: program-ordered before
                    # this batch's softmax chain so its matmuls (and PSUM
                    # slot) fill the PE idle during the serial chain
                    do_pass2(b - 1)
                partb = spool.tile([128, 1], F32, tag="partb")
                nc.vector.tensor_reduce(partb[:, :], expP[:, :], axis=AX.X, op=OP.add)
                d_ps = mps.tile([1, 1], F32, tag="misc")
                nc.tensor.matmul(
                    d_ps[:, :], ones_col[:, :], partb[:, :], start=True, stop=True
                )
                rd = spool.tile([1, 1], F32, tag="rd")
                nc.vector.reciprocal(rd[:, :], d_ps[:, :])
                r_ps = mps.tile([128, 1], F32, tag="misc")
                nc.tensor.matmul(
                    r_ps[:, :], ones_row[:, :], rd[:, :], start=True, stop=True
                )
                rrepb = spool.tile([128, 1], F32, tag="rrepb")
                nc.vector.tensor_copy(rrepb[:, :], r_ps[:, :])
                nc.vector.tensor_scalar_mul(
                    alpha_r[:, b * TPB : (b + 1) * TPB], expP[:, :], rrepb[:, :]
                )
            do_pass2(BSH - 1)

            nc.sync.dma_start(out=alpha_out.ap(), in_=alpha_r[:, :].bitcast(F32))

    nc.compile()
    return nc


_NC = None


def _get_nc():
    global _NC
    if _NC is None:
        _NC = _build()
    return _NC


def kernel(enc_out, dec_hidden, W_enc, b_enc, W_dec, b_dec, W_full, b_full=None):
    enc_out = np.ascontiguousarray(enc_out, dtype=np.float32)
    dec_hidden = np.ascontiguousarray(dec_hidden, dtype=np.float32)
    in_maps = []
    for c in range(NCORES):
        in_maps.append({
            "enc": enc_out[c * BSH : (c + 1) * BSH],
            "dec": dec_hidden[c * BSH : (c + 1) * BSH],
            "w_enc": np.ascontiguousarray(W_enc, np.float32),
            "b_enc": np.ascontiguousarray(b_enc, np.float32),
            "w_dec": np.ascontiguousarray(W_dec, np.float32),
            "b_dec": np.ascontiguousarray(b_dec, np.float32),
            "w_full": np.ascontiguousarray(W_full, np.float32),
            "ident_in": np.eye(128, dtype=np.float32).astype("bfloat16")
            if hasattr(np, "bfloat16") else __import__("ml_dtypes").bfloat16(np.eye(128, dtype=np.float32)),
        })
    res = bass_utils.run_bass_kernel_spmd(_get_nc(), in_maps, core_ids=list(range(NCORES)))
    awe = np.concatenate([res.results[c]["awe"] for c in range(NCORES)], axis=0)
    alpha = np.concatenate(
        [
            res.results[c]["alpha_raw"]
            .reshape(128, BSH, TPB)
            .transpose(1, 2, 0)
            .reshape(BSH, L)
            for c in range(NCORES)
        ],
        axis=0,
    )
    return awe, alpha
